# revision 30
# baseline (speedup 1.0000x reference)
"""CrossNetMix (DCN-V2 mixture-of-low-rank-experts) Trainium2 kernel.

Strategy: data-parallel over batch across 8 cores (2048 rows each), tensors
kept feature-major on chip ([d, b]) so every matmul contraction lands on SBUF
partitions. All matmul operands are bf16 (fp32 PSUM accumulation) — this
enables fast weight load so LDWEIGHTS hides behind the matmul stream, and
halves HBM traffic. The host pre-packs weights/activations partition-major so
every DMA is contiguous per partition.

Per layer (fused), per 512-row batch chunk ("slot"):
  g_rep = xi @ Wg4              -> gating, output replicated at partition
                                   bases 0/32/64/96 (for row-tiled broadcast)
  expg  = exp(g_rep)            -> ACT
  sums/rb                       -> partition-sum and 1/sum broadcast via tiny
                                   matmuls; wsb = expg * rb  (softmax)
  h1 = tanh(xi @ Vflat)         -> [er=512, b]
  h2 = tanh(blockdiag_C @ h1)   -> per-expert C folded into 128x128 pairs
  wb_mc = sel_mc.T @ wsb_rep    -> gate weights broadcast over ranks; 4
                                   row-tiled matmuls (tile_position) that run
                                   concurrently in the PE array
  y  = h2 * wb
  mixed = Uflat.T @ y           -> [d, b]
  xi = x0 * (mixed + b) + xi    -> fused combine from PSUM

Scheduling notes (these are what the runtime actually rewards):
  - Layer-outer / chunk-inner emission with per-chunk persistent xi tiles:
    the four batch chunks form independent pipelines.
  - The gate chain ping-pongs PE->ACT->PE->DVE->PE; its PE ops are spread
    between V groups (sums after V0, rb after V1, wb mid-C) so each sits
    ~1.7us of queued PE work after the ACT/DVE op it consumes, and the next
    slot's gating matmuls fill the C->U boundary (covering tanh(h2)+y).
  - Combine adds run on GpSimd so the U-phase DVE queue holds only the
    scalar_tensor_tensor evacuations (735ns < 852ns U-group pace) and PSUM
    banks always recycle in time.
  - ~52 dummy matmuls on a memset scratch tile bridge the ~12us DMA head so
    HAM has unthrottled the PE clock (1.2->2.4GHz) before real work lands.

Measured: ~250us HW exec (vs 351.7us baseline), rel err ~3.5e-3 (gate 2e-2).
PE 91.5% active, median matmul issue gap at the 216ns roofline.
"""

import sys

import numpy as np
from ml_dtypes import bfloat16

if "/opt/trn_rl_repo" not in sys.path:
    sys.path.insert(0, "/opt/trn_rl_repo")

import concourse.bass as bass
import concourse.bacc as bacc
import concourse.mybir as mybir
from concourse.tile import TileContext
from concourse.bass_utils import run_bass_kernel_spmd

AF = mybir.ActivationFunctionType
OP = mybir.AluOpType
F32 = mybir.dt.float32
BF16 = mybir.dt.bfloat16

N_CROSS = 3
E = 8            # experts
D = 1024         # feature dim
R = 64           # low rank
B = 16384        # full batch
NCORES = 8
BC = B // NCORES  # rows per core
CHUNK = 512       # batch tile (matmul free dim)
NCHUNK = BC // CHUNK
P = 128
KC = D // P       # d-chunks
ER = E * R        # 512
MC = ER // P      # (e,r)-chunks


def _build():
    nc = bacc.Bacc(None)
    Xp = nc.declare_dram_parameter("Xp", [P, NCHUNK, KC, CHUNK], BF16, isOutput=False)
    Vp = nc.declare_dram_parameter("Vp", [P, N_CROSS, KC, ER], BF16, isOutput=False)
    Cp = nc.declare_dram_parameter("Cp", [P, N_CROSS, MC, P], BF16, isOutput=False)
    Up = nc.declare_dram_parameter("Up", [P, N_CROSS, MC, D], BF16, isOutput=False)
    Wgp = nc.declare_dram_parameter("Wgp", [P, KC, P], BF16, isOutput=False)
    SelWB = nc.declare_dram_parameter("SelWB", [P, P], BF16, isOutput=False)
    OnesC = nc.declare_dram_parameter("OnesC", [P, 1], BF16, isOutput=False)
    OnesR = nc.declare_dram_parameter("OnesR", [1, P], BF16, isOutput=False)
    Bp = nc.declare_dram_parameter("Bp", [P, N_CROSS, KC], F32, isOutput=False)
    OutT = nc.declare_dram_parameter("OutT", [NCHUNK, KC, P, CHUNK], BF16, isOutput=True)

    with TileContext(nc) as tc:
        with (
            tc.sbuf_pool(name="wpool", bufs=1) as wpool,
            tc.sbuf_pool(name="xpool", bufs=1) as xpool,
            tc.sbuf_pool(name="xipool", bufs=1) as xipool,
            tc.sbuf_pool(name="h1pool", bufs=8) as h1pool,
            tc.sbuf_pool(name="h2pool", bufs=3) as h2pool,
            tc.sbuf_pool(name="ypool", bufs=8) as ypool,
            tc.sbuf_pool(name="tpool", bufs=6) as tpool,
            tc.sbuf_pool(name="spool", bufs=3) as spool,
            tc.psum_pool(name="psmm", bufs=3) as psmm,
            tc.psum_pool(name="psu", bufs=2) as psu,
            tc.psum_pool(name="pswb", bufs=2) as pswb,
            tc.psum_pool(name="psg", bufs=1) as psg,
        ):
            # ---- resident weights / selectors ----
            wg_sb = wpool.tile([P, KC, P], BF16)
            nc.sync.dma_start(wg_sb, Wgp[:])
            selwb_sb = wpool.tile([P, P], BF16)
            nc.sync.dma_start(selwb_sb, SelWB[:])
            onesc_sb = wpool.tile([P, 1], BF16)
            nc.sync.dma_start(onesc_sb, OnesC[:])
            onesr_sb = wpool.tile([1, P], BF16)
            nc.sync.dma_start(onesr_sb, OnesR[:])
            b_sb = wpool.tile([P, N_CROSS, KC], F32)
            nc.sync.dma_start(b_sb, Bp[:])

            v_sb = wpool.tile([P, N_CROSS, KC, ER], BF16)
            u_sb = wpool.tile([P, N_CROSS, MC, D], BF16)
            c_sb = wpool.tile([P, N_CROSS, MC, P], BF16)

            # x0: split chunk 0 finely so layer-0 compute starts ASAP
            x0s = []
            for c in range(NCHUNK):
                t = xpool.tile([P, KC, CHUNK], BF16, tag=f"x0_{c}")
                x0s.append(t)
            for q in range(4):
                sl = slice(q * (KC // 4), (q + 1) * (KC // 4))
                nc.sync.dma_start(x0s[0][:, sl], Xp[:, 0, sl])
            # layer-0 V weights per kc (matches the accumulation order)
            for kc in range(KC):
                nc.sync.dma_start(v_sb[:, 0, kc, :], Vp[:, 0, kc, :])
            nc.sync.dma_start(c_sb[:, 0], Cp[:, 0])
            for c in range(1, NCHUNK):
                nc.sync.dma_start(x0s[c], Xp[:, c])
            for mc in range(MC):
                nc.sync.dma_start(u_sb[:, 0, mc, :], Up[:, 0, mc, :])
            for i in range(1, N_CROSS):
                for kc in range(KC):
                    nc.sync.dma_start(v_sb[:, i, kc, :], Vp[:, i, kc, :])
                nc.sync.dma_start(c_sb[:, i], Cp[:, i])
                for mc in range(MC):
                    nc.sync.dma_start(u_sb[:, i, mc, :], Up[:, i, mc, :])

            xis = [
                xipool.tile([P, KC, CHUNK], BF16, tag=f"xi_{c}", name=f"xi_{c}")
                for c in range(NCHUNK)
            ]

            # PE warm-up: HAM un-throttles only after ~3.4us of sustained PE
            # activity, and the input DMA feed takes ~13us to deliver the
            # first tiles. Spin dummy matmuls on a memset scratch tile so the
            # real matmuls start at full clock.
            wsc = wpool.tile([P, 256], BF16, name="warm_scratch")
            nc.vector.memset(wsc, 0.0)
            wps_ = psmm.tile([P, 256], F32, tag="mm", name="warm_ps")
            for _ in range(52):
                nc.tensor.matmul(wps_, wsc[:, 0:128], wsc, start=True, stop=True)

            # The softmax gate chain for slot s+1 ping-pongs PE -> ACT -> PE
            # -> DVE -> PE. Emit it in three pieces spread across slot s so
            # every PE op of the chain sits several microseconds of queued
            # work after the ACT/DVE op it consumes:
            #   A (slot top):    gps matmuls + exp
            #   B (after V):     sums matmul + reciprocal + cast
            #   C (after C/y):   rb broadcast matmul + wsb multiply
            def chain_a(i, c):
                src = x0s[c] if i == 0 else xis[c]
                gps = psg.tile([P, CHUNK], F32, tag="g", name=f"gps_{i}_{c}")
                for kc in range(KC):
                    nc.tensor.matmul(
                        gps,
                        wg_sb[:, kc, :],
                        src[:, kc, :],
                        start=(kc == 0),
                        stop=(kc == KC - 1),
                    )
                expg = spool.tile([P, CHUNK], BF16, tag="expg", name=f"expg_{i}_{c}")
                nc.scalar.activation(expg, gps, AF.Exp)
                return {"i": i, "c": c, "expg": expg}

            def chain_b(st):
                i, c = st["i"], st["c"]
                sums = psg.tile([1, CHUNK], F32, tag="g", name=f"sums_{i}_{c}")
                nc.tensor.matmul(sums, onesc_sb, st["expg"], start=True, stop=True)
                rfast = spool.tile([1, CHUNK], F32, tag="rfast", name=f"rf_{i}_{c}")
                nc.vector.reciprocal_approx_fast(rfast, sums)
                rrow = spool.tile([1, CHUNK], BF16, tag="rrow", name=f"rr_{i}_{c}")
                nc.vector.tensor_copy(rrow, rfast)
                st["rrow"] = rrow

            def chain_c(st):
                i, c = st["i"], st["c"]
                rb = psg.tile([P, CHUNK], F32, tag="g", name=f"rb_{i}_{c}")
                nc.tensor.matmul(rb, onesr_sb, st["rrow"], start=True, stop=True)
                wsb = spool.tile([P, CHUNK], BF16, tag="wsb", name=f"wsb_{i}_{c}")
                nc.vector.tensor_tensor(wsb, st["expg"], rb, OP.mult)
                return wsb

            slots = [(i, c) for i in range(N_CROSS) for c in range(NCHUNK)]

            def v_group(i, c, src, mc):
                vps = psmm.tile([P, CHUNK], F32, tag="mm")
                for kc in range(KC):
                    nc.tensor.matmul(
                        vps,
                        v_sb[:, i, kc, mc * P : (mc + 1) * P],
                        src[:, kc, :],
                        start=(kc == 0),
                        stop=(kc == KC - 1),
                    )
                h1 = h1pool.tile([P, CHUNK], BF16, tag="h1")
                nc.scalar.activation(h1, vps, AF.Tanh)
                return h1

            def c_group(i, c, mc, h1):
                cps = psmm.tile([P, CHUNK], F32, tag="mm")
                nc.tensor.matmul(
                    cps, c_sb[:, i, mc, :], h1, start=True, stop=True
                )
                h2 = h2pool.tile([P, CHUNK], BF16, tag="h2")
                nc.scalar.activation(h2, cps, AF.Tanh)
                return h2

            def wb_mm(i, c, wsb, mc):
                wbp = pswb.tile([P, CHUNK], F32, tag="wb", name=f"wb_{i}_{c}_{mc}")
                nc.tensor.matmul(
                    wbp,
                    selwb_sb[32 * mc : 32 * mc + 8, :],
                    wsb[32 * mc : 32 * mc + 8, :],
                    start=True,
                    stop=True,
                    tile_position=(32 * mc, 0),
                )
                return wbp

            # Slot layout: the gate chain's PE ops are interleaved between
            # V groups so each sits ~1.7us of queued PE work after the
            # ACT/DVE op that feeds it; the next slot's gating matmuls fill
            # the C->U boundary (covering the last tanh + y multiply).
            st_cur = chain_a(*slots[0])
            for si, (i, c) in enumerate(slots):
                x0 = x0s[c]
                xi = xis[c]
                src = x0 if i == 0 else xi
                h1s = []
                h1s.append(v_group(i, c, src, 0))
                chain_b(st_cur)                       # sums (exp long done)
                h1s.append(v_group(i, c, src, 1))
                wsb = chain_c(st_cur)                 # rb + wsb multiply
                h1s.append(v_group(i, c, src, 2))
                h1s.append(v_group(i, c, src, 3))
                # C stage; wb matmuls interleave between C matmuls so (a)
                # the last C group's tanh has PE work in front of it and (b)
                # wb2/wb3 issue after the y-mults of wb0/wb1 have had time to
                # free their PSUM banks (pswb rotates 2 banks)
                h2s = [c_group(i, c, 0, h1s[0]), c_group(i, c, 1, h1s[1])]
                wbs = [wb_mm(i, c, wsb, 0), wb_mm(i, c, wsb, 1)]
                h2s.append(c_group(i, c, 2, h1s[2]))
                wbs.append(wb_mm(i, c, wsb, 2))
                h2s.append(c_group(i, c, 3, h1s[3]))
                wbs.append(wb_mm(i, c, wsb, 3))
                ys = []
                for mc in range(MC):
                    y = ypool.tile([P, CHUNK], BF16, tag="y")
                    nc.vector.tensor_tensor(y, h2s[mc], wbs[mc], OP.mult)
                    ys.append(y)
                # next slot's gating matmuls cover the tanh(h2)/y latency
                # before the U stage needs y
                if si + 1 < len(slots):
                    st_cur = chain_a(*slots[si + 1])
                else:
                    for _ in range(8):
                        nc.tensor.matmul(
                            wps_, wsc[:, 0:128], wsc, start=True, stop=True
                        )
                # ---- U stage + fused combine ----
                for dc in range(KC):
                    ups = psu.tile([P, CHUNK], F32, tag="u")
                    for mc in range(MC):
                        nc.tensor.matmul(
                            ups,
                            u_sb[:, i, mc, dc * P : (dc + 1) * P],
                            ys[mc],
                            start=(mc == 0),
                            stop=(mc == MC - 1),
                        )
                    tmp = tpool.tile([P, CHUNK], BF16, tag="tmp")
                    nc.vector.scalar_tensor_tensor(
                        tmp,
                        ups,
                        b_sb[:, i, dc : dc + 1],
                        x0[:, dc, :],
                        OP.add,
                        OP.mult,
                    )
                    # adds on GpSimd: the U-phase DVE queue then holds only
                    # the stt evacuations (735ns < 852ns U-group pace), so
                    # PSUM banks always recycle in time. The final slot's
                    # tail adds go on the faster Vector engine instead —
                    # nothing left competes for it and they sit on the
                    # drain's critical path.
                    eng = nc.vector if (si == len(slots) - 1 and dc >= 5) else nc.gpsimd
                    eng.tensor_tensor(
                        xi[:, dc, :], tmp, src[:, dc, :], OP.add
                    )
                    if i == N_CROSS - 1:
                        nc.sync.dma_start(OutT[c, dc], xi[:, dc, :])
    nc.compile()
    return nc


_CTX = {}


def _get_nc():
    if "nc" not in _CTX:
        _CTX["nc"] = _build()
    return _CTX["nc"]


def _prep_weights(U, V, C, Wg, b):
    f = np.float32
    U = np.asarray(U, dtype=f)
    V = np.asarray(V, dtype=f)
    C = np.asarray(C, dtype=f)
    Wg = np.asarray(Wg, dtype=f)
    b = np.asarray(b, dtype=f)
    # Vl[i, d, e*R+r] = V[i, e, d, r]; partition-major: Vp[p, i, kc, m]
    Vl = V.transpose(0, 2, 1, 3).reshape(N_CROSS, D, ER)
    Vp = np.ascontiguousarray(
        Vl.reshape(N_CROSS, KC, P, ER).transpose(2, 0, 1, 3)
    ).astype(bfloat16)
    # Ul[i, e*R+r, d] = U[i, e, d, r]; Up[p, i, mc, d]
    Ul = U.transpose(0, 1, 3, 2).reshape(N_CROSS, ER, D)
    Up = np.ascontiguousarray(
        Ul.reshape(N_CROSS, MC, P, D).transpose(2, 0, 1, 3)
    ).astype(bfloat16)
    # block-diagonal expert pairs for the C stage; Cp[p, i, mc, s]
    Cb = np.zeros((N_CROSS, MC, P, P), dtype=f)
    for i in range(N_CROSS):
        for m in range(MC):
            Cb[i, m, :R, :R] = C[i, 2 * m]
            Cb[i, m, R:, R:] = C[i, 2 * m + 1]
    Cp = np.ascontiguousarray(Cb.transpose(2, 0, 1, 3)).astype(bfloat16)
    # gating weight with output replicated at col offsets 0/32/64/96
    Wg4 = np.zeros((D, P), dtype=f)
    for j in range(4):
        Wg4[:, 32 * j : 32 * j + E] = Wg.T
    Wgp = np.ascontiguousarray(
        Wg4.reshape(KC, P, P).transpose(1, 0, 2)
    ).astype(bfloat16)
    # row-tiled gate-broadcast selectors: rows 32*mc+q -> cols j with
    # q == 2*mc + j//R
    SelWB = np.zeros((P, P), dtype=f)
    for mc in range(MC):
        for j in range(P):
            SelWB[32 * mc + 2 * mc + j // R, j] = 1.0
    SelWB = SelWB.astype(bfloat16)
    OnesC = np.zeros((P, 1), dtype=f)
    OnesC[:E, 0] = 1.0
    OnesC = OnesC.astype(bfloat16)
    OnesR = np.ones((1, P), dtype=f).astype(bfloat16)
    # Bp[p, i, kc] = b[i, kc*P + p]
    Bp = np.ascontiguousarray(b.reshape(N_CROSS, KC, P).transpose(2, 0, 1))
    return dict(
        Vp=Vp, Up=Up, Cp=Cp, Wgp=Wgp, SelWB=SelWB, OnesC=OnesC, OnesR=OnesR, Bp=Bp
    )


def kernel(x, U, V, C, Wg, b, _trace=False):
    nc = _get_nc()
    w = _prep_weights(U, V, C, Wg, b)
    xs = np.asarray(x, dtype=np.float32).reshape(NCORES, BC, D)
    in_maps = []
    for ci in range(NCORES):
        # Xp[p, c, kc, j] = x[c*CHUNK + j, kc*P + p]
        xc = xs[ci].reshape(NCHUNK, CHUNK, KC, P).transpose(3, 0, 2, 1)
        m = {"Xp": np.ascontiguousarray(xc).astype(bfloat16)}
        m.update(w)
        in_maps.append(m)
    res = run_bass_kernel_spmd(nc, in_maps, list(range(NCORES)), trace=_trace)
    kernel.last_result = res
    outs = []
    for ci in range(NCORES):
        o = np.asarray(res.results[ci]["OutT"]).astype(np.float32)
        # OutT[c, kc, p, j] -> [c*CHUNK + j, kc*P + p]
        outs.append(o.transpose(0, 3, 1, 2).reshape(BC, D))
    out = np.concatenate(outs, axis=0)
    return np.ascontiguousarray(out, dtype=np.float32)
